# revision 1
# baseline (speedup 1.0000x reference)
"""Trainium2 Bass kernel for a first-order IIR low-pass filter (v2b).

y_t = alpha * x_t + (1 - alpha) * y_{t-1},  y_{-1} = 0
x: [16, 65536, 64] float32  ->  y: [16, 65536, 64] float32

Same structure as v2 (contiguous time-block DMA layout + PE state
matmuls + DVE scan), with the input path in bf16:

  - the host pre-scales x by alpha and stores bf16, so the scan runs
    directly in the output domain (y_t = beta*y_{t-1} + x'_t) and no
    final alpha-scale pass is needed;
  - input HBM traffic halves (bf16), output stays f32.

Input quantization is ~2^-9 relative on x'; through the filter this
bounds |err|/max|y| at ~2e-3, well under the 2e-2 gate.

Sharding (8 cores): 4 batch-groups of 4 batches x 2 time-halves of
32768 steps, each staged with a 128-step halo (zeros for the first
half).
"""

import math
import os
import sys

import numpy as np

try:
    import concourse.bass as bass
except ImportError:
    sys.path.insert(0, "/opt/trn_rl_repo")
    import concourse.bass as bass

import concourse.bacc as bacc
import concourse.mybir as mybir
import concourse.tile as tile
import ml_dtypes
from concourse import bass_utils

SAMPLE_RATE = 16000
CUTOFF_FREQ = 1000.0
_DT = 1.0 / SAMPLE_RATE
_TAU = 1.0 / (2.0 * math.pi * CUTOFF_FREQ)
ALPHA = _DT / (_DT + _TAU)
BETA = 1.0 - ALPHA

B, T, C = 16, 65536, 64
N_CORES = 8
BG = 4                    # batches per core
TH = T // 2               # timesteps per core
SD = int(os.environ.get("IIR_SD", "16"))   # timesteps per partition block
PT = 128 * SD             # timesteps per tile
NT = TH // PT             # tiles per core
HALO = 128                # history window feeding the state matmul

DT_IN = mybir.dt.bfloat16
XBUFS = int(os.environ.get("IIR_XBUFS", "3"))
YBUFS = int(os.environ.get("IIR_YBUFS", "3"))
PSBUFS = int(os.environ.get("IIR_PSBUFS", "4"))

_cached_nc = None


def _w_matrices():
    """Stationary weights for the state matmuls (lhsT layout [k, m]).

    W_s[k, m] = beta^(SD*(m-k) - 1 - s)  for m > k else 0
    H[k, m]  = beta^(127 - k + SD*m)
    (the scan runs on alpha-prescaled inputs, so no alpha here)
    """
    k = np.arange(128, dtype=np.float64)[:, None]
    m = np.arange(128, dtype=np.float64)[None, :]
    ws = np.zeros((SD, 128, 128), np.float64)
    for s in range(SD):
        e = SD * (m - k) - 1 - s
        ws[s] = np.where(e >= 0, BETA ** np.maximum(e, 0.0), 0.0)
    h = BETA ** (127.0 - k + SD * m)
    ws[np.abs(ws) < 1e-30] = 0.0
    h[np.abs(h) < 1e-30] = 0.0
    return ws.astype(ml_dtypes.bfloat16), h.astype(ml_dtypes.bfloat16)


def _build_program():
    nc = bacc.Bacc("TRN2", target_bir_lowering=False, debug=False)

    x_in = nc.dram_tensor("x", [BG, HALO + TH, C], DT_IN, kind="ExternalInput").ap()
    a_w = nc.dram_tensor("a_w", [SD, 128, 128], DT_IN, kind="ExternalInput").ap()
    a_h = nc.dram_tensor("a_h", [128, 128], DT_IN, kind="ExternalInput").ap()
    y_out = nc.dram_tensor(
        "y", [BG, TH, C], mybir.dt.float32, kind="ExternalOutput"
    ).ap()

    mult = mybir.AluOpType.mult
    add = mybir.AluOpType.add

    with tile.TileContext(nc) as tc:
        with (
            tc.tile_pool(name="w", bufs=1) as wpool,
            tc.tile_pool(name="xin", bufs=XBUFS) as xpool,
            tc.tile_pool(name="hin", bufs=XBUFS) as hpool,
            tc.tile_pool(name="ysc", bufs=YBUFS) as ypool,
            tc.tile_pool(name="ps", bufs=PSBUFS, space="PSUM") as pspool,
        ):
            wt = wpool.tile([128, SD, 128], DT_IN, tag="wt")
            nc.sync.dma_start(wt[:], a_w.rearrange("s k m -> k s m"))
            hw = wpool.tile([128, 128], DT_IN, tag="hw")
            nc.sync.dma_start(hw[:], a_h[:])

            for j in range(NT):
                # [partition = SD-step block, batch, step, channel]; per
                # (p, b) the (step, channel) run is contiguous in DRAM.
                xt = xpool.tile([128, BG, SD, C], DT_IN, tag="xt")
                src = x_in[:, HALO + j * PT : HALO + (j + 1) * PT, :].rearrange(
                    "b (p s) c -> p b s c", p=128
                )
                nc.sync.dma_start(xt[:], src)

                # halo: the 128 steps before this tile, one per partition
                ht = hpool.tile([128, BG, C], DT_IN, tag="ht")
                hsrc = x_in[:, j * PT : j * PT + HALO, :].rearrange("b k c -> k b c")
                nc.sync.dma_start(ht[:], hsrc)

                # state entering each partition's window
                ps = pspool.tile([128, BG, C], mybir.dt.float32, tag="ps")
                nc.tensor.matmul(ps[:], hw[:], ht[:], start=True, stop=False)
                for s in range(SD):
                    nc.tensor.matmul(
                        ps[:], wt[:, s, :], xt[:, :, s, :],
                        start=False, stop=(s == SD - 1),
                    )

                # local SD-step scan per partition; inputs are
                # alpha-prescaled so this directly produces y (f32)
                yt = ypool.tile([128, BG, SD, C], mybir.dt.float32, tag="yt")
                nc.vector.scalar_tensor_tensor(
                    yt[:, :, 0, :], ps[:], BETA, xt[:, :, 0, :], mult, add
                )
                for s in range(1, SD):
                    nc.vector.scalar_tensor_tensor(
                        yt[:, :, s, :], yt[:, :, s - 1, :], BETA,
                        xt[:, :, s, :], mult, add,
                    )

                dst = y_out[:, j * PT : (j + 1) * PT, :].rearrange(
                    "b (p s) c -> p b s c", p=128
                )
                nc.scalar.dma_start(dst, yt[:])

    nc.compile()
    return nc


def _get_program():
    global _cached_nc
    if _cached_nc is None:
        _cached_nc = _build_program()
    return _cached_nc


def _shard_inputs(x):
    ws, h = _w_matrices()
    consts = {"a_w": ws, "a_h": h}
    xs = (np.float32(ALPHA) * x).astype(ml_dtypes.bfloat16)
    in_maps = []
    for g in range(4):
        for hh in range(2):
            b0 = BG * g
            t0 = TH * hh
            xl = np.empty((BG, HALO + TH, C), ml_dtypes.bfloat16)
            if hh == 0:
                xl[:, :HALO] = 0.0
                xl[:, HALO:] = xs[b0 : b0 + BG, 0:TH]
            else:
                xl[:] = xs[b0 : b0 + BG, t0 - HALO : t0 + TH]
            in_maps.append({"x": xl, **consts})
    return in_maps


def run(x, trace=False):
    x = np.ascontiguousarray(np.asarray(x, dtype=np.float32))
    assert x.shape == (B, T, C), x.shape
    nc = _get_program()
    in_maps = _shard_inputs(x)
    res = bass_utils.run_bass_kernel_spmd(
        nc, in_maps, core_ids=list(range(N_CORES)), trace=trace
    )
    y = np.empty((B, T, C), np.float32)
    core = 0
    for g in range(4):
        for hh in range(2):
            y[BG * g : BG * (g + 1), TH * hh : TH * (hh + 1)] = res.results[core]["y"]
            core += 1
    return y, res


def kernel(x):
    y, _ = run(x, trace=False)
    return y



# revision 5
# speedup vs baseline: 1.5018x; 1.5018x over previous
"""Trainium2 Bass kernel for a first-order IIR low-pass filter (v2b).

y_t = alpha * x_t + (1 - alpha) * y_{t-1},  y_{-1} = 0
x: [16, 65536, 64] float32  ->  y: [16, 65536, 64] float32

Same structure as v2 (contiguous time-block DMA layout + PE state
matmuls + DVE scan), with the input path in bf16:

  - the host pre-scales x by alpha and stores bf16, so the scan runs
    directly in the output domain (y_t = beta*y_{t-1} + x'_t) and no
    final alpha-scale pass is needed;
  - input HBM traffic halves (bf16), output stays f32.

Input quantization is ~2^-9 relative on x'; through the filter this
bounds |err|/max|y| at ~2e-3, well under the 2e-2 gate.

Sharding (8 cores): 4 batch-groups of 4 batches x 2 time-halves of
32768 steps, each staged with a 128-step halo (zeros for the first
half).
"""

import math
import os
import sys

import numpy as np

try:
    import concourse.bass as bass
except ImportError:
    sys.path.insert(0, "/opt/trn_rl_repo")
    import concourse.bass as bass

import concourse.bacc as bacc
import concourse.mybir as mybir
import concourse.tile as tile
import ml_dtypes
from concourse import bass_utils

SAMPLE_RATE = 16000
CUTOFF_FREQ = 1000.0
_DT = 1.0 / SAMPLE_RATE
_TAU = 1.0 / (2.0 * math.pi * CUTOFF_FREQ)
ALPHA = _DT / (_DT + _TAU)
BETA = 1.0 - ALPHA

B, T, C = 16, 65536, 64
N_CORES = 8
BG = 4                    # batches per core
TH = T // 2               # timesteps per core
SD = int(os.environ.get("IIR_SD", "16"))   # timesteps per partition block
PT = 128 * SD             # timesteps per tile
NT = TH // PT             # tiles per core
HALO = 128                # history window feeding the state matmul

DT_IN = mybir.dt.bfloat16
XBUFS = int(os.environ.get("IIR_XBUFS", "4"))
YBUFS = int(os.environ.get("IIR_YBUFS", "4"))
PSBUFS = int(os.environ.get("IIR_PSBUFS", "4"))
# output dtype: bf16 on the wire (host upcasts); IIR_YF32=1 restores f32
Y_F32 = os.environ.get("IIR_YF32", "0") == "1"
DT_OUT = mybir.dt.float32 if Y_F32 else mybir.dt.bfloat16
NP_OUT = np.float32 if Y_F32 else ml_dtypes.bfloat16

_cached_nc = None


def _w_matrices():
    """Stationary weights for the state matmuls (lhsT layout [k, m]).

    W_s[k, m] = beta^(SD*(m-k) - 1 - s)  for m > k else 0
    H[k, m]  = beta^(127 - k + SD*m)
    (the scan runs on alpha-prescaled inputs, so no alpha here)
    """
    k = np.arange(128, dtype=np.float64)[:, None]
    m = np.arange(128, dtype=np.float64)[None, :]
    ws = np.zeros((SD, 128, 128), np.float64)
    for s in range(SD):
        e = SD * (m - k) - 1 - s
        ws[s] = np.where(e >= 0, BETA ** np.maximum(e, 0.0), 0.0)
    h = BETA ** (127.0 - k + SD * m)
    ws[np.abs(ws) < 1e-30] = 0.0
    h[np.abs(h) < 1e-30] = 0.0
    return ws.astype(ml_dtypes.bfloat16), h.astype(ml_dtypes.bfloat16)


def _build_program():
    nc = bacc.Bacc("TRN2", target_bir_lowering=False, debug=False)

    x_in = nc.dram_tensor("x", [BG, HALO + TH, C], DT_IN, kind="ExternalInput").ap()
    a_w = nc.dram_tensor("a_w", [SD, 128, 128], DT_IN, kind="ExternalInput").ap()
    a_h = nc.dram_tensor("a_h", [128, 128], DT_IN, kind="ExternalInput").ap()
    y_out = nc.dram_tensor("y", [BG, TH, C], DT_OUT, kind="ExternalOutput").ap()

    mult = mybir.AluOpType.mult
    add = mybir.AluOpType.add

    with tile.TileContext(nc) as tc:
        with (
            tc.tile_pool(name="w", bufs=1) as wpool,
            tc.tile_pool(name="xin", bufs=XBUFS) as xpool,
            tc.tile_pool(name="hin", bufs=XBUFS) as hpool,
            tc.tile_pool(name="ysc", bufs=YBUFS) as ypool,
            tc.tile_pool(name="ps", bufs=PSBUFS, space="PSUM") as pspool,
        ):
            wt = wpool.tile([128, SD, 128], DT_IN, tag="wt")
            nc.sync.dma_start(wt[:], a_w.rearrange("s k m -> k s m"))
            hw = wpool.tile([128, 128], DT_IN, tag="hw")
            nc.sync.dma_start(hw[:], a_h[:])

            for j in range(NT):
                # [partition = SD-step block, batch, step, channel]; per
                # (p, b) the (step, channel) run is contiguous in DRAM.
                xt = xpool.tile([128, BG, SD, C], DT_IN, tag="xt")
                src = x_in[:, HALO + j * PT : HALO + (j + 1) * PT, :].rearrange(
                    "b (p s) c -> p b s c", p=128
                )
                nc.sync.dma_start(xt[:], src)

                # halo: the 128 steps before this tile, one per partition
                ht = hpool.tile([128, BG, C], DT_IN, tag="ht")
                hsrc = x_in[:, j * PT : j * PT + HALO, :].rearrange("b k c -> k b c")
                nc.sync.dma_start(ht[:], hsrc)

                # state entering each partition's window
                ps = pspool.tile([128, BG, C], mybir.dt.float32, tag="ps")
                nc.tensor.matmul(ps[:], hw[:], ht[:], start=True, stop=False)
                for s in range(SD):
                    nc.tensor.matmul(
                        ps[:], wt[:, s, :], xt[:, :, s, :],
                        start=False, stop=(s == SD - 1),
                    )

                # local SD-step scan per partition; inputs are
                # alpha-prescaled so this directly produces y
                yt = ypool.tile([128, BG, SD, C], DT_OUT, tag="yt")
                nc.vector.scalar_tensor_tensor(
                    yt[:, :, 0, :], ps[:], BETA, xt[:, :, 0, :], mult, add
                )
                for s in range(1, SD):
                    nc.vector.scalar_tensor_tensor(
                        yt[:, :, s, :], yt[:, :, s - 1, :], BETA,
                        xt[:, :, s, :], mult, add,
                    )

                dst = y_out[:, j * PT : (j + 1) * PT, :].rearrange(
                    "b (p s) c -> p b s c", p=128
                )
                nc.scalar.dma_start(dst, yt[:])

    nc.compile()
    return nc


def _get_program():
    global _cached_nc
    if _cached_nc is None:
        _cached_nc = _build_program()
    return _cached_nc


def _shard_inputs(x):
    ws, h = _w_matrices()
    consts = {"a_w": ws, "a_h": h}
    xs = (np.float32(ALPHA) * x).astype(ml_dtypes.bfloat16)
    in_maps = []
    for g in range(4):
        for hh in range(2):
            b0 = BG * g
            t0 = TH * hh
            xl = np.empty((BG, HALO + TH, C), ml_dtypes.bfloat16)
            if hh == 0:
                xl[:, :HALO] = 0.0
                xl[:, HALO:] = xs[b0 : b0 + BG, 0:TH]
            else:
                xl[:] = xs[b0 : b0 + BG, t0 - HALO : t0 + TH]
            in_maps.append({"x": xl, **consts})
    return in_maps


def run(x, trace=False):
    x = np.ascontiguousarray(np.asarray(x, dtype=np.float32))
    assert x.shape == (B, T, C), x.shape
    nc = _get_program()
    in_maps = _shard_inputs(x)
    res = bass_utils.run_bass_kernel_spmd(
        nc, in_maps, core_ids=list(range(N_CORES)), trace=trace
    )
    y = np.empty((B, T, C), np.float32)
    core = 0
    for g in range(4):
        for hh in range(2):
            y[BG * g : BG * (g + 1), TH * hh : TH * (hh + 1)] = res.results[core][
                "y"
            ].astype(np.float32)
            core += 1
    return y, res


def kernel(x):
    y, _ = run(x, trace=False)
    return y

